# revision 17
# baseline (speedup 1.0000x reference)
"""Multi-head attention (softmax over the query axis) on 8 trn2 cores.

Sharding: tensor-parallel over heads — 2 heads per core. Each core computes
its heads' projections + attention + a partial output projection (row-parallel
Wo); the host sums the 8 partial outputs and adds bo.

Device-side layout choices (host pre-packs everything):
  - activations are shipped TRANSPOSED (d on partitions) as bf16, so every
    matmul contracts over the partition dim with natural-layout DMAs.
  - scores are computed transposed ([t, s]) so the softmax axis (query s) is
    the free axis.
  - 1/rowsum is folded into V's rows (16K elems) instead of the attention
    matrix (4.2M elems).

Schedule (v2):
  - all weights packed dt-major and streamed in chunks interleaved with the
    activation strips, so the first projection matmul starts as early as
    possible and no weight load sits on the P-Q/P-K critical DMA window.
  - psum-bank drains (bias adds) moved off the Scalar engine onto
    GpSimd/Vector; Scalar does exp only.
  - exp row-sums moved off the ACT accumulator onto GpSimd tensor_reduce.
  - phase B fuses scores-h1 + pass2-h0 (c-major over resident h0 exps) +
    pass2-h1 (i-major, one-strip lag, 3-deep exp ring) in exactly 8 psum
    banks, absorbing the old serial S2 phases.
  - O phase: h-outer stationary reuse, double-buffered [128,1024] psum
    tiles per output half, output DMA streams during the matmuls.
"""

import json

import numpy as np
import ml_dtypes

import concourse.bass as bass
import concourse.mybir as mybir
import concourse.tile as tile
from concourse import bass_utils

BF16 = mybir.dt.bfloat16
F32 = mybir.dt.float32
AF = mybir.ActivationFunctionType
ALU = mybir.AluOpType
AX = mybir.AxisListType

N_CORES = 8
H = 16
D = 2048
DK = 128
S = 2048
HPC = H // N_CORES          # heads per core = 2
NT = D // 128               # 16 tiles along d / t
NSC = S // 512              # 4 chunks of 512 along s / m
SCALE = 1.0 / float(np.sqrt(DK))

TRACE = False
LAST_RESULTS = None
PHASE_MARKS = []


def _mark(nc, label):
    PHASE_MARKS.append((label, nc.next_id()))


# The walrus in this container accepts only ONE sem-wait per instruction
# (setupSyncWait: "Too many sync wait commands"), but Tile attaches one wait
# per depended-on semaphore. Split extra waits onto single-wait NoOps inserted
# just before the instruction on the same engine, at BIR-JSON level so every
# compile path (native + bass2jax/axon) is covered.
def _split_multi_waits(raw: bytes) -> bytes:
    m = json.loads(raw)
    ctr = 0
    changed = False
    for fn in m.get("functions", []):
        for blk in fn.get("blocks", []):
            insts = blk.get("instructions", [])
            out = []
            for inst in insts:
                si = inst.get("sync_info")
                waits = (si.get("on_wait") or []) if si else []
                if len(waits) > 1:
                    changed = True
                    for w in waits[:-1]:
                        ctr += 1
                        out.append(
                            {
                                "debug": inst.get("debug"),
                                "engine": inst["engine"],
                                "ins": [],
                                "name": f"I-wsplit-{ctr}",
                                "opcode": "NoOp",
                                "outs": [],
                                "sync_info": {"on_update": [], "on_wait": [w]},
                            }
                        )
                    si["on_wait"] = [waits[-1]]
                out.append(inst)
            if changed:
                blk["instructions"] = out
    if not changed:
        return raw
    return json.dumps(m).encode()


_orig_to_json_bytes = bass.Bass.to_json_bytes


def _to_json_bytes_split(self):
    return _split_multi_waits(_orig_to_json_bytes(self))


bass.Bass.to_json_bytes = _to_json_bytes_split


def _build_bass():
    nc = bass.Bass(trn_type="TRN2")

    qT = nc.dram_tensor("qT", [D, S], BF16, kind="ExternalInput")
    kT = nc.dram_tensor("kT", [D, S], BF16, kind="ExternalInput")
    vT = nc.dram_tensor("vT", [D, S], BF16, kind="ExternalInput")
    # all three projection weight stacks packed dt-major:
    # col = (dt*HPC + h)*128 + k, row = d within the dt strip
    wq = nc.dram_tensor("wq", [128, NT * HPC * 128], BF16, kind="ExternalInput")
    wk = nc.dram_tensor("wk", [128, NT * HPC * 128], BF16, kind="ExternalInput")
    wv2 = nc.dram_tensor("wv2", [128, NT * HPC * 128], BF16, kind="ExternalInput")
    wo = nc.dram_tensor("wo", [128, HPC * D], BF16, kind="ExternalInput")
    bqk = nc.dram_tensor("bqk", [128, 2 * HPC], F32, kind="ExternalInput")
    bvb = nc.dram_tensor("bvb", [128, HPC * 128], F32, kind="ExternalInput")
    out = nc.dram_tensor("out_p", [S, D], BF16, kind="ExternalOutput")

    WCH = NT * HPC * 128 // 4  # weight chunk: 4 dt strips = [128, 1024]

    with tile.TileContext(nc) as tc:
        with (
            tc.tile_pool(name="wpool", bufs=1) as wpool,
            tc.tile_pool(name="acts", bufs=1) as acts,
            tc.tile_pool(name="xpool", bufs=4) as xpool,
            tc.tile_pool(name="small", bufs=2) as small,
            tc.tile_pool(name="opool", bufs=2) as opool,
            tc.tile_pool(name="exppool", bufs=1) as exppool,
        ):
            # --- resident weights ---
            wq_sb = wpool.tile([128, NT * HPC * 128], BF16)
            wk_sb = wpool.tile([128, NT * HPC * 128], BF16)
            wv2_sb = wpool.tile([128, NT * HPC * 128], BF16)
            wo_sb = wpool.tile([128, HPC * D], BF16)
            bqk_sb = wpool.tile([128, 2 * HPC], F32)
            bvb_sb = wpool.tile([128, HPC * 128], F32)

            # --- resident per-head activations ---
            QT = [acts.tile([128, S], BF16, name=f"QT{h}") for h in range(HPC)]
            KT = [acts.tile([128, S], BF16, name=f"KT{h}") for h in range(HPC)]
            V = [acts.tile([128, NT * 128], BF16, name=f"V{h}") for h in range(HPC)]
            HT = [acts.tile([128, S], BF16, name=f"HT{h}") for h in range(HPC)]
            vsca = [
                acts.tile([128, NT * 128], BF16, name=f"vsca{h}") for h in range(HPC)
            ]
            # h0's exp strips persist through phase B; h1 uses a 3-deep ring
            EXP0 = [
                exppool.tile([128, S], BF16, name=f"exp0_{i}", tag=f"exp0_{i}", bufs=1)
                for i in range(NT)
            ]

            # ---------------- phases P-Q / P-K: Q^T / K^T projections ----------
            # Per-head psum pools (4 banks each) so phase-A pools map onto
            # per-head release zones. DMA emission order is the schedule:
            # xs strips interleaved with weight chunks.
            with (
                tc.tile_pool(name="pq0", bufs=1, space="PSUM") as pq0,
                tc.tile_pool(name="pq1", bufs=1, space="PSUM") as pq1,
            ):
                pqp = [pq0, pq1]
                for xdram, w_sb, dst, bcol, label in (
                    (qT, wq_sb, QT, 0, "P-Q"),
                    (kT, wk_sb, KT, HPC, "P-K"),
                ):
                    _mark(nc, label)
                    is_q = xdram is qT
                    ps = [
                        [
                            pqp[h].tile(
                                [128, 512], F32, name=f"pp{h}{c}", tag=f"pp{h}{c}",
                                bufs=1,
                            )
                            for c in range(NSC)
                        ]
                        for h in range(HPC)
                    ]
                    for dt in range(NT):
                        xs = xpool.tile([128, S], BF16, name="xs", tag="xs", bufs=6)
                        nc.sync.dma_start(xs[:], xdram[dt * 128 : (dt + 1) * 128, :])
                        if is_q:
                            # wq chunks interleave with the q-strip stream;
                            # tiny bqk rides along late
                            if dt < 4:
                                nc.sync.dma_start(
                                    wq_sb[:, dt * WCH : (dt + 1) * WCH],
                                    wq[:, dt * WCH : (dt + 1) * WCH],
                                )
                            elif dt == 8:
                                nc.sync.dma_start(bqk_sb[:], bqk[:])
                            elif dt == NT - 1:
                                # wk chunk 0 must precede the first k strip
                                nc.sync.dma_start(
                                    wk_sb[:, 0:WCH], wk[:, 0:WCH]
                                )
                        else:
                            # remaining wk chunks just ahead of their strips;
                            # wv2 chunks ride the late k-strip stream
                            if dt % 4 == 0 and dt < 12:
                                j = dt // 4 + 1
                                nc.sync.dma_start(
                                    wk_sb[:, j * WCH : (j + 1) * WCH],
                                    wk[:, j * WCH : (j + 1) * WCH],
                                )
                            elif dt >= 12:
                                j = dt - 12
                                nc.sync.dma_start(
                                    wv2_sb[:, j * WCH : (j + 1) * WCH],
                                    wv2[:, j * WCH : (j + 1) * WCH],
                                )
                        for h in range(HPC):
                            for c in range(NSC):
                                nc.tensor.matmul(
                                    ps[h][c][:],
                                    w_sb[:, (dt * HPC + h) * 128 : (dt * HPC + h + 1) * 128],
                                    xs[:, c * 512 : (c + 1) * 512],
                                    start=(dt == 0),
                                    stop=(dt == NT - 1),
                                )
                            if dt == NT - 1:
                                # drains emitted right after this head's last
                                # matmul, split vector+scalar, so h0's psum
                                # zone frees while h1's matmuls still run
                                for c in range(NSC):
                                    if c % 2 == 0:
                                        nc.vector.tensor_scalar_add(
                                            dst[h][:, c * 512 : (c + 1) * 512],
                                            ps[h][c][:],
                                            bqk_sb[:, bcol + h : bcol + h + 1],
                                        )
                                    else:
                                        nc.scalar.activation(
                                            dst[h][:, c * 512 : (c + 1) * 512],
                                            ps[h][c][:],
                                            AF.Identity,
                                            bias=bqk_sb[:, bcol + h : bcol + h + 1],
                                            scale=1.0,
                                        )

            # ---------------- phase A: V projection + scores/exp h0 ----------
            # pscA allocated first -> lands on pq0's (h0) zone; psv second ->
            # pq1's (h1) zone.
            with (
                tc.tile_pool(name="pscA", bufs=1, space="PSUM") as pscA,
                tc.tile_pool(name="ppsv", bufs=1, space="PSUM") as ppsv,
            ):
                _mark(nc, "A")
                nc.sync.dma_start(bvb_sb[:], bvb[:])
                nc.sync.dma_start(wo_sb[:], wo[:])

                rects = {}

                def emit_scores_full(i, expt):
                    # h0 strips: full [128,2048] psum (4 banks), ONE exp with
                    # fused row-sum accumulator
                    psc = pscA.tile([128, S], F32, name=f"pscf{i}", tag="pscf", bufs=1)
                    for c in range(NSC):
                        nc.tensor.matmul(
                            psc[:, c * 512 : (c + 1) * 512],
                            KT[0][:, i * 128 : (i + 1) * 128],
                            QT[0][:, c * 512 : (c + 1) * 512],
                            start=True,
                            stop=True,
                        )
                    sumt = small.tile([128, 1], F32, name="sumt", tag="sum", bufs=4)
                    nc.scalar.activation(
                        expt[:],
                        psc[:],
                        AF.Exp,
                        scale=SCALE,
                        accum_out=sumt[:],
                    )
                    rect = small.tile(
                        [128, 1], F32, name="rect", tag=f"rec{i % 4}", bufs=2
                    )
                    rects[(0, i)] = rect
                    nc.vector.reciprocal(rect[:], sumt[:])

                def emit_scores_b(i, expt):
                    # h1 strips: two [128,1024] halves through DOUBLE-TAGGED
                    # psum (psc0/psc1) so half1's matmuls overlap half0's exp;
                    # row-sum off ACT onto one full-strip DVE reduce
                    for half in range(2):
                        psc = pscB.tile(
                            [128, 1024], F32, name=f"pscB{half}", tag=f"pscB{half}",
                            bufs=1,
                        )
                        for cc in range(2):
                            c = half * 2 + cc
                            nc.tensor.matmul(
                                psc[:, cc * 512 : (cc + 1) * 512],
                                KT[1][:, i * 128 : (i + 1) * 128],
                                QT[1][:, c * 512 : (c + 1) * 512],
                                start=True,
                                stop=True,
                            )
                        nc.scalar.activation(
                            expt[:, half * 1024 : (half + 1) * 1024],
                            psc[:],
                            AF.Exp,
                            scale=SCALE,
                        )
                    rect = small.tile(
                        [128, 1], F32, name="rect", tag=f"rec{i % 4}", bufs=2
                    )
                    rects[(1, i)] = rect
                    nc.vector.reduce_sum(rect[:], expt[:], axis=AX.X)
                    nc.vector.reciprocal(rect[:], rect[:])

                def emit_vscale(h, i):
                    nc.vector.tensor_scalar_mul(
                        vsca[h][:, i * 128 : (i + 1) * 128],
                        V[h][:, i * 128 : (i + 1) * 128],
                        rects[(h, i)][:],
                    )

                # V groups interleaved with score strips at dt-pack
                # granularity so the PE never idles on the single-buffered
                # full-strip psc (in-order engine queue).
                for g in range(4):
                    psv = [
                        ppsv.tile(
                            [128, 512], F32, name=f"psv{tt}", tag=f"psv{tt}", bufs=1
                        )
                        for tt in range(4)
                    ]
                    for q in range(4):
                        emit_scores_full(4 * g + q, EXP0[4 * g + q])
                        for dt in range(4 * q, 4 * q + 4):
                            xc = xpool.tile(
                                [128, 512], BF16, name="xc", tag="xc", bufs=12
                            )
                            # issue from the otherwise-idle GpSimd queue: the
                            # Sync engine's serial descriptor issue (~0.65us
                            # each x64) was pacing this phase
                            nc.gpsimd.dma_start(
                                xc[:],
                                vT[dt * 128 : (dt + 1) * 128, g * 512 : (g + 1) * 512],
                            )
                            for tt in range(4):
                                nc.tensor.matmul(
                                    psv[tt][:, : HPC * 128],
                                    xc[:, tt * 128 : (tt + 1) * 128],
                                    wv2_sb[:, dt * HPC * 128 : (dt + 1) * HPC * 128],
                                    start=(dt == 0),
                                    stop=(dt == NT - 1),
                                )
                    for tt in range(4):
                        t_tile = g * 4 + tt
                        for h in range(HPC):
                            nc.vector.tensor_tensor(
                                V[h][:, t_tile * 128 : (t_tile + 1) * 128],
                                psv[tt][:, h * 128 : (h + 1) * 128],
                                bvb_sb[:, h * 128 : (h + 1) * 128],
                                op=ALU.add,
                            )
                    for i in range(4 * g, 4 * g + 4):
                        emit_vscale(0, i)

            # ---------------- phase B: scores/exp h1 + pass2 h1 --------------
            # Banks: pscB0/pscB1 [128,1024] (4) + ph1 4x[128,512] (4) = 8.
            # ACT-paced at ~2.7us/strip; pass2-h1 rides under the exp shadow
            # with a one-strip lag through the 3-deep eh1 ring.
            with (
                tc.tile_pool(name="ppscB", bufs=1, space="PSUM") as pscB,
                tc.tile_pool(name="pph1", bufs=1, space="PSUM") as pph1,
            ):
                _mark(nc, "B")
                ph1 = [
                    pph1.tile([128, 512], F32, name=f"ph1{c}", tag=f"ph1{c}", bufs=1)
                    for c in range(NSC)
                ]
                eh1 = {}

                def emit_pass2_h1(i):
                    expt_i = eh1.pop(i)
                    for c in range(NSC):
                        nc.tensor.matmul(
                            ph1[c][:],
                            vsca[1][:, i * 128 : (i + 1) * 128],
                            expt_i[:, c * 512 : (c + 1) * 512],
                            start=(i == 0),
                            stop=(i == NT - 1),
                        )

                for i in range(NT):
                    expt = exppool.tile(
                        [128, S], BF16, name=f"eh1_{i}", tag="eh1", bufs=3
                    )
                    eh1[i] = expt
                    emit_scores_b(i, expt)
                    emit_vscale(1, i)
                    if i > 0:
                        emit_pass2_h1(i - 1)
                emit_pass2_h1(NT - 1)
                for c in range(NSC):
                    if c % 2 == 0:
                        nc.vector.tensor_copy(
                            HT[1][:, c * 512 : (c + 1) * 512], ph1[c][:]
                        )
                    else:
                        nc.scalar.copy(HT[1][:, c * 512 : (c + 1) * 512], ph1[c][:])

            # ------- phase C: pass2 h0 (c-major over resident EXP0) + O ------
            # Banks: ph0 bufs=2 (2) + po0/po1 [128,1024] (4) = 6. The O chunk
            # for s-range c lags pass2-h0 chunk c by one step so the PE never
            # waits on the HT0 copies.
            with (
                tc.tile_pool(name="pph0", bufs=1, space="PSUM") as pph0,
                tc.tile_pool(name="ppo", bufs=1, space="PSUM") as ppo,
            ):
                _mark(nc, "C")

                def emit_p2h0_chunk(c):
                    ph0_tile = pph0.tile(
                        [128, 512], F32, name=f"ph0{c}", tag="ph0", bufs=2
                    )
                    for j in range(NT):
                        nc.tensor.matmul(
                            ph0_tile[:],
                            vsca[0][:, j * 128 : (j + 1) * 128],
                            EXP0[j][:, c * 512 : (c + 1) * 512],
                            start=(j == 0),
                            stop=(j == NT - 1),
                        )
                    nc.vector.tensor_copy(
                        HT[0][:, c * 512 : (c + 1) * 512], ph0_tile[:]
                    )

                def emit_o_chunk(c):
                    for st in range(4 * c, 4 * c + 4):
                        po = [
                            ppo.tile(
                                [128, 1024], F32, name=f"po{cp}", tag=f"po{cp}",
                                bufs=1,
                            )
                            for cp in range(2)
                        ]
                        for h in range(HPC):
                            for cp in range(2):
                                for cc in range(2):
                                    nc.tensor.matmul(
                                        po[cp][:, cc * 512 : (cc + 1) * 512],
                                        HT[h][:, st * 128 : (st + 1) * 128],
                                        wo_sb[
                                            :,
                                            h * D + cp * 1024 + cc * 512 : h * D
                                            + cp * 1024
                                            + (cc + 1) * 512,
                                        ],
                                        start=(h == 0),
                                        stop=(h == HPC - 1),
                                    )
                        # merged [128,2048] out tile + single DMA per st
                        ot = opool.tile([128, 2048], BF16, name="ot", tag="ot", bufs=2)
                        nc.vector.tensor_copy(ot[:, 0:1024], po[0][:])
                        nc.scalar.copy(ot[:, 1024:2048], po[1][:])
                        nc.sync.dma_start(out[st * 128 : (st + 1) * 128, :], ot[:])

                emit_p2h0_chunk(0)
                for c in range(1, NSC):
                    emit_p2h0_chunk(c)
                    emit_o_chunk(c - 1)
                emit_o_chunk(NSC - 1)

    return nc


_NC = None


def _get_nc():
    global _NC
    if _NC is None:
        _NC = _build_bass()
    return _NC


def _prep_inputs(query, key, value, Wq, bq, Wk, bk, Wv, bv, Wo, bo):
    """Host-side shard + pack. Returns per-core input maps."""
    bf = ml_dtypes.bfloat16
    f32 = np.float32

    query = np.asarray(query, f32)
    key = np.asarray(key, f32)
    value = np.asarray(value, f32)
    Wq = np.asarray(Wq, f32)
    Wk = np.asarray(Wk, f32)
    Wv = np.asarray(Wv, f32)
    Wo = np.asarray(Wo, f32)
    bq = np.asarray(bq, f32)
    bk = np.asarray(bk, f32)
    bv = np.asarray(bv, f32)

    qT = np.ascontiguousarray(query.T).astype(bf)
    kT = np.ascontiguousarray(key.T).astype(bf)
    vT = np.ascontiguousarray(value.T).astype(bf)

    in_maps = []
    for c in range(N_CORES):
        heads = [c * HPC + j for j in range(HPC)]

        # dt-major packing for all three stacks:
        # col = (dt*HPC + h)*128 + k, row = d within tile
        def pack_w(W):
            return np.concatenate(
                [
                    np.concatenate(
                        [W[hh].reshape(NT, 128, DK)[dt] for hh in heads], axis=1
                    )
                    for dt in range(NT)
                ],
                axis=1,
            ).astype(bf)

        wo_p = np.concatenate(
            [Wo[hh * DK : (hh + 1) * DK, :] for hh in heads], axis=1
        ).astype(bf)

        bqk = np.stack(
            [bq[hh] for hh in heads] + [bk[hh] for hh in heads], axis=1
        ).astype(f32)
        bvb = np.concatenate(
            [np.broadcast_to(bv[hh][None, :], (128, DK)) for hh in heads], axis=1
        ).astype(f32)

        in_maps.append(
            {
                "qT": qT,
                "kT": kT,
                "vT": vT,
                "wq": np.ascontiguousarray(pack_w(Wq)),
                "wk": np.ascontiguousarray(pack_w(Wk)),
                "wv2": np.ascontiguousarray(pack_w(Wv)),
                "wo": np.ascontiguousarray(wo_p),
                "bqk": np.ascontiguousarray(bqk),
                "bvb": np.ascontiguousarray(bvb),
            }
        )
    return in_maps


def kernel(query, key, value, Wq, bq, Wk, bk, Wv, bv, Wo, bo):
    global LAST_RESULTS
    in_maps = _prep_inputs(query, key, value, Wq, bq, Wk, bk, Wv, bv, Wo, bo)
    nc = _get_nc()
    res = bass_utils.run_bass_kernel_spmd(
        nc, in_maps, core_ids=list(range(N_CORES)), trace=TRACE
    )
    LAST_RESULTS = res
    acc = res.results[0]["out_p"].astype(np.float32)
    for c in range(1, N_CORES):
        acc += res.results[c]["out_p"].astype(np.float32)
    acc += np.asarray(bo, np.float32)[None, :]
    return acc


# revision 18
# speedup vs baseline: 1.0675x; 1.0675x over previous
"""Multi-head attention (softmax over the query axis) on 8 trn2 cores.

Sharding: tensor-parallel over heads — 2 heads per core. Each core computes
its heads' projections + attention + a partial output projection (row-parallel
Wo); the host sums the 8 partial outputs and adds bo.

Device-side layout choices (host pre-packs everything):
  - activations are shipped TRANSPOSED (d on partitions) as bf16; vT is
    additionally blocked [128, NT, S] so ONE dma descriptor fetches four
    d-strips of a t-chunk (descriptor issue rate was pacing the V phase).
  - scores are computed transposed ([t, s]) so the softmax axis (query s) is
    the free axis.
  - 1/rowsum is folded into V's rows (16K elems) instead of the attention
    matrix (4.2M elems).
  - exp tiles are shared between heads: h1's exp strip i overwrites h0's
    once pass2-h0 consumed it (WAR through the tile tag).

Schedule (v6):
  - weights packed dt-major, streamed in chunks interleaved with the
    activation strips (first projection matmul starts ~2us after DMA ramp).
  - psum drains (bias adds) split across Vector+Scalar, emitted inside the
    final d-strip so the next phase's psum zone frees early.
  - phase A: V-projection packs interleaved between h0 score strips;
    full-strip [128,2048] score psum -> ONE exp per strip (saves the second
    ACTIVATE's fixed overhead) with fused row-sum accumulator.
  - phase B: pass2-h0 (i-major, under h1's exp shadow) + scores-h1
    (double-tagged [128,1024] psum halves) — ACT-paced.
  - phase D: pass2-h1, PE-dense.
  - phase O: h-outer stationary reuse, merged [128,2048] out tiles, single
    DMA per row-strip.
"""

import json

import numpy as np
import ml_dtypes

import concourse.bass as bass
import concourse.mybir as mybir
import concourse.tile as tile
from concourse import bass_utils

BF16 = mybir.dt.bfloat16
F32 = mybir.dt.float32
AF = mybir.ActivationFunctionType
ALU = mybir.AluOpType
AX = mybir.AxisListType

N_CORES = 8
H = 16
D = 2048
DK = 128
S = 2048
HPC = H // N_CORES          # heads per core = 2
NT = D // 128               # 16 tiles along d / t
NSC = S // 512              # 4 chunks of 512 along s / m
SCALE = 1.0 / float(np.sqrt(DK))

TRACE = False
LAST_RESULTS = None
PHASE_MARKS = []


def _mark(nc, label):
    PHASE_MARKS.append((label, nc.next_id()))


# The walrus in this container accepts only ONE sem-wait per instruction
# (setupSyncWait: "Too many sync wait commands"), but Tile attaches one wait
# per depended-on semaphore. Split extra waits onto single-wait NoOps inserted
# just before the instruction on the same engine, at BIR-JSON level so every
# compile path (native + bass2jax/axon) is covered.
def _split_multi_waits(raw: bytes) -> bytes:
    m = json.loads(raw)
    ctr = 0
    changed = False
    for fn in m.get("functions", []):
        for blk in fn.get("blocks", []):
            insts = blk.get("instructions", [])
            out = []
            for inst in insts:
                si = inst.get("sync_info")
                waits = (si.get("on_wait") or []) if si else []
                if len(waits) > 1:
                    changed = True
                    for w in waits[:-1]:
                        ctr += 1
                        out.append(
                            {
                                "debug": inst.get("debug"),
                                "engine": inst["engine"],
                                "ins": [],
                                "name": f"I-wsplit-{ctr}",
                                "opcode": "NoOp",
                                "outs": [],
                                "sync_info": {"on_update": [], "on_wait": [w]},
                            }
                        )
                    si["on_wait"] = [waits[-1]]
                out.append(inst)
            if changed:
                blk["instructions"] = out
    if not changed:
        return raw
    return json.dumps(m).encode()


_orig_to_json_bytes = bass.Bass.to_json_bytes


def _to_json_bytes_split(self):
    return _split_multi_waits(_orig_to_json_bytes(self))


bass.Bass.to_json_bytes = _to_json_bytes_split


def _build_bass():
    nc = bass.Bass(trn_type="TRN2")

    qT = nc.dram_tensor("qT", [D, S], BF16, kind="ExternalInput")
    kT = nc.dram_tensor("kT", [D, S], BF16, kind="ExternalInput")
    # vT blocked: vT3[p, dt, s] = value[s, dt*128+p]
    vT3 = nc.dram_tensor("vT3", [128, NT, S], BF16, kind="ExternalInput")
    # projection weight stacks packed dt-major:
    # col = (dt*HPC + h)*128 + k, row = d within the dt strip
    wq = nc.dram_tensor("wq", [128, NT * HPC * 128], BF16, kind="ExternalInput")
    wk = nc.dram_tensor("wk", [128, NT * HPC * 128], BF16, kind="ExternalInput")
    wv2 = nc.dram_tensor("wv2", [128, NT * HPC * 128], BF16, kind="ExternalInput")
    wo = nc.dram_tensor("wo", [128, HPC * D], BF16, kind="ExternalInput")
    bqk = nc.dram_tensor("bqk", [128, 2 * HPC], F32, kind="ExternalInput")
    bvb = nc.dram_tensor("bvb", [128, HPC * 128], F32, kind="ExternalInput")
    out = nc.dram_tensor("out_p", [S, D], BF16, kind="ExternalOutput")

    WCH = NT * HPC * 128 // 4  # weight chunk: 4 dt strips = [128, 1024]

    with tile.TileContext(nc) as tc:
        with (
            tc.tile_pool(name="wpool", bufs=1) as wpool,
            tc.tile_pool(name="acts", bufs=1) as acts,
            tc.tile_pool(name="xpool", bufs=4) as xpool,
            tc.tile_pool(name="small", bufs=2) as small,
            tc.tile_pool(name="opool", bufs=2) as opool,
            tc.tile_pool(name="exppool", bufs=1) as exppool,
        ):
            # --- resident weights ---
            wq_sb = wpool.tile([128, NT * HPC * 128], BF16)
            wk_sb = wpool.tile([128, NT * HPC * 128], BF16)
            wv2_sb = wpool.tile([128, NT * HPC * 128], BF16)
            wo_sb = wpool.tile([128, HPC * D], BF16)
            bqk_sb = wpool.tile([128, 2 * HPC], F32)
            bvb_sb = wpool.tile([128, HPC * 128], F32)

            # --- resident per-head activations ---
            QT = [acts.tile([128, S], BF16, name=f"QT{h}") for h in range(HPC)]
            KT = [acts.tile([128, S], BF16, name=f"KT{h}") for h in range(HPC)]
            V = [acts.tile([128, NT * 128], BF16, name=f"V{h}") for h in range(HPC)]
            HT = [acts.tile([128, S], BF16, name=f"HT{h}") for h in range(HPC)]
            vsca = [
                acts.tile([128, NT * 128], BF16, name=f"vsca{h}") for h in range(HPC)
            ]
            # exp strips, shared between heads (h1 overwrites after pass2-h0
            # consumed strip i)
            def exp_tile(i):
                return exppool.tile(
                    [128, S], BF16, name=f"exp{i}", tag=f"exp{i}", bufs=1
                )

            EXP0 = [exp_tile(i) for i in range(NT)]

            # ---------------- phases P-Q / P-K: Q^T / K^T projections ----------
            with (
                tc.tile_pool(name="pq0", bufs=1, space="PSUM") as pq0,
                tc.tile_pool(name="pq1", bufs=1, space="PSUM") as pq1,
            ):
                pqp = [pq0, pq1]
                for xdram, w_sb, dst, bcol, label in (
                    (qT, wq_sb, QT, 0, "P-Q"),
                    (kT, wk_sb, KT, HPC, "P-K"),
                ):
                    _mark(nc, label)
                    is_q = xdram is qT
                    ps = [
                        [
                            pqp[h].tile(
                                [128, 512], F32, name=f"pp{h}{c}", tag=f"pp{h}{c}",
                                bufs=1,
                            )
                            for c in range(NSC)
                        ]
                        for h in range(HPC)
                    ]
                    for dt in range(NT):
                        xs = xpool.tile([128, S], BF16, name="xs", tag="xs", bufs=6)
                        nc.sync.dma_start(xs[:], xdram[dt * 128 : (dt + 1) * 128, :])
                        if is_q:
                            if dt < 4:
                                nc.sync.dma_start(
                                    wq_sb[:, dt * WCH : (dt + 1) * WCH],
                                    wq[:, dt * WCH : (dt + 1) * WCH],
                                )
                            elif dt == 8:
                                nc.sync.dma_start(bqk_sb[:], bqk[:])
                            elif dt == NT - 1:
                                nc.sync.dma_start(wk_sb[:, 0:WCH], wk[:, 0:WCH])
                        else:
                            if dt % 4 == 0 and dt < 12:
                                j = dt // 4 + 1
                                nc.sync.dma_start(
                                    wk_sb[:, j * WCH : (j + 1) * WCH],
                                    wk[:, j * WCH : (j + 1) * WCH],
                                )
                            elif dt >= 12:
                                j = dt - 12
                                nc.sync.dma_start(
                                    wv2_sb[:, j * WCH : (j + 1) * WCH],
                                    wv2[:, j * WCH : (j + 1) * WCH],
                                )
                        for h in range(HPC):
                            for c in range(NSC):
                                nc.tensor.matmul(
                                    ps[h][c][:],
                                    w_sb[:, (dt * HPC + h) * 128 : (dt * HPC + h + 1) * 128],
                                    xs[:, c * 512 : (c + 1) * 512],
                                    start=(dt == 0),
                                    stop=(dt == NT - 1),
                                )
                            if dt == NT - 1:
                                # drains right after this head's last matmul,
                                # split vector+scalar; h0's zone frees while
                                # h1's matmuls still run
                                for c in range(NSC):
                                    if c % 2 == 0:
                                        nc.vector.tensor_scalar_add(
                                            dst[h][:, c * 512 : (c + 1) * 512],
                                            ps[h][c][:],
                                            bqk_sb[:, bcol + h : bcol + h + 1],
                                        )
                                    else:
                                        nc.scalar.activation(
                                            dst[h][:, c * 512 : (c + 1) * 512],
                                            ps[h][c][:],
                                            AF.Identity,
                                            bias=bqk_sb[:, bcol + h : bcol + h + 1],
                                            scale=1.0,
                                        )

            # ---------------- phase A: V projection + scores/exp h0 ----------
            # pscA first -> lands on pq0's (h0) zone; psv second -> pq1's.
            with (
                tc.tile_pool(name="pscA", bufs=1, space="PSUM") as pscA,
                tc.tile_pool(name="ppsv", bufs=1, space="PSUM") as ppsv,
            ):
                _mark(nc, "A")
                nc.sync.dma_start(bvb_sb[:], bvb[:])
                nc.sync.dma_start(wo_sb[:], wo[:])

                rects = {}

                def emit_scores_full(i, expt):
                    # full [128,2048] psum, ONE exp with fused row-sum accum
                    psc = pscA.tile([128, S], F32, name=f"pscf{i}", tag="pscf", bufs=1)
                    for c in range(NSC):
                        nc.tensor.matmul(
                            psc[:, c * 512 : (c + 1) * 512],
                            KT[0][:, i * 128 : (i + 1) * 128],
                            QT[0][:, c * 512 : (c + 1) * 512],
                            start=True,
                            stop=True,
                        )
                    sumt = small.tile([128, 1], F32, name="sumt", tag="sum", bufs=4)
                    nc.scalar.activation(
                        expt[:], psc[:], AF.Exp, scale=SCALE, accum_out=sumt[:]
                    )
                    rect = small.tile(
                        [128, 1], F32, name="rect", tag=f"rec{i % 4}", bufs=2
                    )
                    rects[(0, i)] = rect
                    nc.vector.reciprocal(rect[:], sumt[:])

                def emit_vscale(h, i):
                    nc.vector.tensor_scalar_mul(
                        vsca[h][:, i * 128 : (i + 1) * 128],
                        V[h][:, i * 128 : (i + 1) * 128],
                        rects[(h, i)][:],
                    )

                # V packs (one xc4 = four d-strips of this t-chunk, a single
                # dma descriptor) interleaved between score strips
                for g in range(4):
                    psv = [
                        ppsv.tile(
                            [128, 512], F32, name=f"psv{tt}", tag=f"psv{tt}", bufs=1
                        )
                        for tt in range(4)
                    ]
                    for q in range(4):
                        emit_scores_full(4 * g + q, EXP0[4 * g + q])
                        xc4 = xpool.tile(
                            [128, 4, 512], BF16, name="xc4", tag="xc4", bufs=3
                        )
                        nc.gpsimd.dma_start(
                            xc4[:],
                            vT3[:, 4 * q : 4 * q + 4, g * 512 : (g + 1) * 512],
                        )
                        for dtl in range(4):
                            dt = 4 * q + dtl
                            for tt in range(4):
                                nc.tensor.matmul(
                                    psv[tt][:, : HPC * 128],
                                    xc4[:, dtl, tt * 128 : (tt + 1) * 128],
                                    wv2_sb[:, dt * HPC * 128 : (dt + 1) * HPC * 128],
                                    start=(dt == 0),
                                    stop=(dt == NT - 1),
                                )
                    for tt in range(4):
                        t_tile = g * 4 + tt
                        for h in range(HPC):
                            nc.vector.tensor_tensor(
                                V[h][:, t_tile * 128 : (t_tile + 1) * 128],
                                psv[tt][:, h * 128 : (h + 1) * 128],
                                bvb_sb[:, h * 128 : (h + 1) * 128],
                                op=ALU.add,
                            )
                    for i in range(4 * g, 4 * g + 4):
                        emit_vscale(0, i)

            # -------- phase B: pass2-h0 (i-major) + scores/exp h1 ------------
            # Banks: ph0 4x[128,512] + pscB0/pscB1 [128,1024] = 8. pass2-h0
            # rides under h1's ACT-paced exp stream; h1's exp strip i reuses
            # EXP0[i] right after pass2-h0 consumed it.
            with (
                tc.tile_pool(name="ppscB", bufs=1, space="PSUM") as pscB,
                tc.tile_pool(name="pph0", bufs=1, space="PSUM") as pph0,
            ):
                _mark(nc, "B")
                ph0 = [
                    pph0.tile([128, 512], F32, name=f"ph0{c}", tag=f"ph0{c}", bufs=1)
                    for c in range(NSC)
                ]
                exp1 = {}

                for i in range(NT):
                    # pass2-h0 strip i (frees EXP0[i] for h1's write)
                    for c in range(NSC):
                        nc.tensor.matmul(
                            ph0[c][:],
                            vsca[0][:, i * 128 : (i + 1) * 128],
                            EXP0[i][:, c * 512 : (c + 1) * 512],
                            start=(i == 0),
                            stop=(i == NT - 1),
                        )
                    # scores-h1 strip i -> exp into the reused tile
                    expt = exp_tile(i)
                    exp1[i] = expt
                    sumt = small.tile([128, 2], F32, name="sumt2", tag="sum2", bufs=4)
                    for half in range(2):
                        psc = pscB.tile(
                            [128, 1024], F32, name=f"pscB{half}", tag=f"pscB{half}",
                            bufs=1,
                        )
                        for cc in range(2):
                            c = half * 2 + cc
                            nc.tensor.matmul(
                                psc[:, cc * 512 : (cc + 1) * 512],
                                KT[1][:, i * 128 : (i + 1) * 128],
                                QT[1][:, c * 512 : (c + 1) * 512],
                                start=True,
                                stop=True,
                            )
                        nc.scalar.activation(
                            expt[:, half * 1024 : (half + 1) * 1024],
                            psc[:],
                            AF.Exp,
                            scale=SCALE,
                            accum_out=sumt[:, half : half + 1],
                        )
                    rect = small.tile(
                        [128, 1], F32, name="rect1", tag=f"rc1{i % 4}", bufs=2
                    )
                    nc.vector.reduce_sum(rect[:], sumt[:], axis=AX.X)
                    nc.vector.reciprocal(rect[:], rect[:])
                    rects[(1, i)] = rect
                    emit_vscale(1, i)
                for c in range(NSC):
                    if c % 2 == 0:
                        nc.vector.tensor_copy(
                            HT[0][:, c * 512 : (c + 1) * 512], ph0[c][:]
                        )
                    else:
                        nc.scalar.copy(HT[0][:, c * 512 : (c + 1) * 512], ph0[c][:])

            # ---------------- phase D: pass2-h1, PE-dense --------------------
            with tc.tile_pool(name="pph1", bufs=1, space="PSUM") as pph1:
                _mark(nc, "D")
                ph1 = [
                    pph1.tile([128, 512], F32, name=f"ph1{c}", tag=f"ph1{c}", bufs=1)
                    for c in range(NSC)
                ]
                for i in range(NT):
                    for c in range(NSC):
                        nc.tensor.matmul(
                            ph1[c][:],
                            vsca[1][:, i * 128 : (i + 1) * 128],
                            exp1[i][:, c * 512 : (c + 1) * 512],
                            start=(i == 0),
                            stop=(i == NT - 1),
                        )
                for c in range(NSC):
                    if c % 2 == 0:
                        nc.vector.tensor_copy(
                            HT[1][:, c * 512 : (c + 1) * 512], ph1[c][:]
                        )
                    else:
                        nc.scalar.copy(HT[1][:, c * 512 : (c + 1) * 512], ph1[c][:])

            # ---------------- phase O: partial output projection --------------
            with tc.tile_pool(name="ppo", bufs=1, space="PSUM") as ppo:
                _mark(nc, "O")
                for st in range(NT):
                    po = [
                        ppo.tile(
                            [128, 1024], F32, name=f"po{cp}", tag=f"po{cp}", bufs=2
                        )
                        for cp in range(2)
                    ]
                    for h in range(HPC):
                        # stationary HT[h][st] reused across 4 matmuls
                        for cp in range(2):
                            for cc in range(2):
                                nc.tensor.matmul(
                                    po[cp][:, cc * 512 : (cc + 1) * 512],
                                    HT[h][:, st * 128 : (st + 1) * 128],
                                    wo_sb[
                                        :,
                                        h * D + cp * 1024 + cc * 512 : h * D
                                        + cp * 1024
                                        + (cc + 1) * 512,
                                    ],
                                    start=(h == 0),
                                    stop=(h == HPC - 1),
                                )
                    # merged [128,2048] out tile + a single DMA per row strip
                    ot = opool.tile([128, 2048], BF16, name="ot", tag="ot", bufs=2)
                    nc.vector.tensor_copy(ot[:, 0:1024], po[0][:])
                    nc.scalar.copy(ot[:, 1024:2048], po[1][:])
                    nc.sync.dma_start(out[st * 128 : (st + 1) * 128, :], ot[:])

    return nc


_NC = None


def _get_nc():
    global _NC
    if _NC is None:
        _NC = _build_bass()
    return _NC


def _prep_inputs(query, key, value, Wq, bq, Wk, bk, Wv, bv, Wo, bo):
    """Host-side shard + pack. Returns per-core input maps."""
    bf = ml_dtypes.bfloat16
    f32 = np.float32

    query = np.asarray(query, f32)
    key = np.asarray(key, f32)
    value = np.asarray(value, f32)
    Wq = np.asarray(Wq, f32)
    Wk = np.asarray(Wk, f32)
    Wv = np.asarray(Wv, f32)
    Wo = np.asarray(Wo, f32)
    bq = np.asarray(bq, f32)
    bk = np.asarray(bk, f32)
    bv = np.asarray(bv, f32)

    qT = np.ascontiguousarray(query.T).astype(bf)
    kT = np.ascontiguousarray(key.T).astype(bf)
    # vT3[p, dt, s] = value[s, dt*128+p]
    vT3 = np.ascontiguousarray(
        value.T.reshape(NT, 128, S).transpose(1, 0, 2)
    ).astype(bf)

    in_maps = []
    for c in range(N_CORES):
        heads = [c * HPC + j for j in range(HPC)]

        # dt-major packing: col = (dt*HPC + h)*128 + k, row = d within tile
        def pack_w(W):
            return np.concatenate(
                [
                    np.concatenate(
                        [W[hh].reshape(NT, 128, DK)[dt] for hh in heads], axis=1
                    )
                    for dt in range(NT)
                ],
                axis=1,
            ).astype(bf)

        wo_p = np.concatenate(
            [Wo[hh * DK : (hh + 1) * DK, :] for hh in heads], axis=1
        ).astype(bf)

        bqk = np.stack(
            [bq[hh] for hh in heads] + [bk[hh] for hh in heads], axis=1
        ).astype(f32)
        bvb = np.concatenate(
            [np.broadcast_to(bv[hh][None, :], (128, DK)) for hh in heads], axis=1
        ).astype(f32)

        in_maps.append(
            {
                "qT": qT,
                "kT": kT,
                "vT3": vT3,
                "wq": np.ascontiguousarray(pack_w(Wq)),
                "wk": np.ascontiguousarray(pack_w(Wk)),
                "wv2": np.ascontiguousarray(pack_w(Wv)),
                "wo": np.ascontiguousarray(wo_p),
                "bqk": np.ascontiguousarray(bqk),
                "bvb": np.ascontiguousarray(bvb),
            }
        )
    return in_maps


def kernel(query, key, value, Wq, bq, Wk, bk, Wv, bv, Wo, bo):
    global LAST_RESULTS
    in_maps = _prep_inputs(query, key, value, Wq, bq, Wk, bk, Wv, bv, Wo, bo)
    nc = _get_nc()
    res = bass_utils.run_bass_kernel_spmd(
        nc, in_maps, core_ids=list(range(N_CORES)), trace=TRACE
    )
    LAST_RESULTS = res
    acc = res.results[0]["out_p"].astype(np.float32)
    for c in range(1, N_CORES):
        acc += res.results[c]["out_p"].astype(np.float32)
    acc += np.asarray(bo, np.float32)[None, :]
    return acc


# revision 26
# speedup vs baseline: 1.1634x; 1.0898x over previous
"""Multi-head attention (softmax over the query axis) on 8 trn2 cores.

Sharding: tensor-parallel over heads — 2 heads per core. Each core computes
its heads' projections + attention + a partial output projection (row-parallel
Wo); the host sums the 8 partial outputs and adds bo.

Device-side layout choices (host pre-packs everything):
  - activations are shipped TRANSPOSED (d on partitions) as bf16, so every
    matmul contracts over the partition dim with natural-layout DMAs.
  - scores are computed transposed ([t, s]) so the softmax axis (query s) is
    the free axis: one exp-activation per strip with fused row-sum.
  - 1/rowsum is folded into V's rows (16K elems) instead of the attention
    matrix (4.2M elems).
"""

import json

import numpy as np
import ml_dtypes

import concourse.bass as bass
import concourse.mybir as mybir
import concourse.tile as tile
from concourse import bass_utils

BF16 = mybir.dt.bfloat16
F32 = mybir.dt.float32
AF = mybir.ActivationFunctionType

N_CORES = 8
H = 16
D = 2048
DK = 128
S = 2048
HPC = H // N_CORES          # heads per core = 2
NT = D // 128               # 16 tiles along d / t
NSC = S // 512              # 4 chunks of 512 along s / m
SCALE = 1.0 / float(np.sqrt(DK))

TRACE = False
LAST_RESULTS = None
PHASE_MARKS = []


def _mark(nc, label):
    PHASE_MARKS.append((label, nc.next_id()))


# The walrus in this container accepts only ONE sem-wait per instruction
# (setupSyncWait: "Too many sync wait commands"), but Tile attaches one wait
# per depended-on semaphore. Split extra waits onto single-wait NoOps inserted
# just before the instruction on the same engine, at BIR-JSON level so every
# compile path (native + bass2jax/axon) is covered.
def _split_multi_waits(raw: bytes) -> bytes:
    m = json.loads(raw)
    ctr = 0
    changed = False
    for fn in m.get("functions", []):
        for blk in fn.get("blocks", []):
            insts = blk.get("instructions", [])
            out = []
            for inst in insts:
                si = inst.get("sync_info")
                waits = (si.get("on_wait") or []) if si else []
                if len(waits) > 1:
                    changed = True
                    for w in waits[:-1]:
                        ctr += 1
                        out.append(
                            {
                                "debug": inst.get("debug"),
                                "engine": inst["engine"],
                                "ins": [],
                                "name": f"I-wsplit-{ctr}",
                                "opcode": "NoOp",
                                "outs": [],
                                "sync_info": {"on_update": [], "on_wait": [w]},
                            }
                        )
                    si["on_wait"] = [waits[-1]]
                out.append(inst)
            if changed:
                blk["instructions"] = out
    if not changed:
        return raw
    return json.dumps(m).encode()


_orig_to_json_bytes = bass.Bass.to_json_bytes


def _to_json_bytes_split(self):
    return _split_multi_waits(_orig_to_json_bytes(self))


bass.Bass.to_json_bytes = _to_json_bytes_split


def _build_bass(loop_n=None):
    nc = bass.Bass(trn_type="TRN2")

    qT = nc.dram_tensor("qT", [D, S], BF16, kind="ExternalInput")
    kT = nc.dram_tensor("kT", [D, S], BF16, kind="ExternalInput")
    # vT blocked [128, NT, S]: vT3[p, dt, s] = value[s, dt*128+p], so one
    # descriptor fetches four d-strips of a t-chunk
    vT3 = nc.dram_tensor("vT3", [128, NT, S], BF16, kind="ExternalInput")
    # wq/wk packed dt-major like wv2: col = (dt*HPC + h)*128 + k
    wq = nc.dram_tensor("wq", [128, HPC * NT * 128], BF16, kind="ExternalInput")
    wk = nc.dram_tensor("wk", [128, HPC * NT * 128], BF16, kind="ExternalInput")
    wv2 = nc.dram_tensor("wv2", [128, NT * HPC * 128], BF16, kind="ExternalInput")
    wo = nc.dram_tensor("wo", [128, HPC * D], BF16, kind="ExternalInput")
    bqk = nc.dram_tensor("bqk", [128, 2 * HPC], F32, kind="ExternalInput")
    bvb = nc.dram_tensor("bvb", [128, HPC * 128], F32, kind="ExternalInput")
    out = nc.dram_tensor("out_p", [S, D], BF16, kind="ExternalOutput")

    with tile.TileContext(nc) as tc:
        with (
            tc.tile_pool(name="wpool", bufs=1) as wpool,
            tc.tile_pool(name="acts", bufs=1) as acts,
            tc.tile_pool(name="xpool", bufs=4) as xpool,
            tc.tile_pool(name="small", bufs=2) as small,
            tc.tile_pool(name="opool", bufs=2) as opool,
            tc.tile_pool(name="exppool", bufs=1) as exppool,
        ):
            # --- resident weights ---
            wq_sb = wpool.tile([128, HPC * NT * 128], BF16)
            wk_sb = wpool.tile([128, HPC * NT * 128], BF16)
            wv2_sb = wpool.tile([128, NT * HPC * 128], BF16)
            wo_sb = wpool.tile([128, HPC * D], BF16)
            bqk_sb = wpool.tile([128, 2 * HPC], F32)
            bvb_sb = wpool.tile([128, HPC * 128], F32)
            WCH = NT * HPC * 128 // 4  # weight chunk: 4 dt strips

            # --- resident per-head activations ---
            QT = [acts.tile([128, S], BF16, name=f"QT{h}") for h in range(HPC)]
            KT = [acts.tile([128, S], BF16, name=f"KT{h}") for h in range(HPC)]
            V = [acts.tile([128, NT * 128], BF16, name=f"V{h}") for h in range(HPC)]
            HT = [acts.tile([128, S], BF16, name=f"HT{h}") for h in range(HPC)]

            # benchmark mode: run the whole body loop_n times in one NEFF to
            # amortize dispatch overhead; weights loaded once up front.
            loop_ctx = None
            if loop_n:
                nc.sync.dma_start(wk_sb[:], wk[:])
                nc.sync.dma_start(wv2_sb[:], wv2[:])
                nc.sync.dma_start(bvb_sb[:], bvb[:])
                nc.sync.dma_start(wo_sb[:], wo[:])
                loop_ctx = tc.For_i(0, loop_n, 1)
                loop_ctx.__enter__()

            # ---------------- phase P-QK: Q^T / K^T projections ---------------
            # Full-strip loads ([128, S] = 4KB lines); one psum bank per
            # (head, s-chunk), accumulated across all 16 d-strips.
            with tc.tile_pool(name="ppqk", bufs=1, space="PSUM") as ppqk:
                for xdram, w_sb, dst, bcol in ((qT, wq_sb, QT, 0), (kT, wk_sb, KT, HPC)):
                    _mark(nc, "P-Q" if xdram is qT else "P-K")
                    is_q = xdram is qT
                    ps = [
                        [
                            ppqk.tile(
                                [128, 512], F32, name=f"pp{h}{c}", tag=f"pp{h}{c}", bufs=1
                            )
                            for c in range(NSC)
                        ]
                        for h in range(HPC)
                    ]
                    for dt in range(NT):
                        xs = xpool.tile([128, S], BF16, name="xs", tag="xs", bufs=6)
                        nc.sync.dma_start(xs[:], xdram[dt * 128 : (dt + 1) * 128, :])
                        # weight chunks interleave with the activation strips:
                        # wq rides the first q strips, wk the next, wv2 rides
                        # the late k strips; tiny bqk in the gap
                        if is_q:
                            if dt < 4:
                                nc.sync.dma_start(
                                    wq_sb[:, dt * WCH : (dt + 1) * WCH],
                                    wq[:, dt * WCH : (dt + 1) * WCH],
                                )
                            elif dt < 8:
                                j = dt - 4
                                nc.sync.dma_start(
                                    wk_sb[:, j * WCH : (j + 1) * WCH],
                                    wk[:, j * WCH : (j + 1) * WCH],
                                )
                            elif dt == 8:
                                nc.sync.dma_start(bqk_sb[:], bqk[:])
                        elif dt >= 12:
                            j = dt - 12
                            nc.sync.dma_start(
                                wv2_sb[:, j * WCH : (j + 1) * WCH],
                                wv2[:, j * WCH : (j + 1) * WCH],
                            )
                        for h in range(HPC):
                            for c in range(NSC):
                                nc.tensor.matmul(
                                    ps[h][c][:],
                                    w_sb[:, (dt * HPC + h) * 128 : (dt * HPC + h + 1) * 128],
                                    xs[:, c * 512 : (c + 1) * 512],
                                    start=(dt == 0),
                                    stop=(dt == NT - 1),
                                )
                    for h in range(HPC):
                        for c in range(NSC):
                            nc.scalar.activation(
                                dst[h][:, c * 512 : (c + 1) * 512],
                                ps[h][c][:],
                                AF.Identity,
                                bias=bqk_sb[:, bcol + h : bcol + h + 1],
                                scale=1.0,
                            )

            # ------- phases P-V and S interleaved on one psum pool -----------
            # P-V shares the ph* psum tags (1 bank each) with S pass 2; the V
            # matmul groups are interleaved with S-h0 pass 1 so the PE stays
            # busy under pass 1's ACT-bound exp stream.
            with tc.tile_pool(name="pps", bufs=1, space="PSUM") as pps:
                if not loop_n:
                    nc.sync.dma_start(bvb_sb[:], bvb[:])
                    nc.sync.dma_start(wo_sb[:], wo[:])
                vsca = [
                    small.tile([128, NT * 128], BF16, name=f"vsca{h}", tag=f"vsca{h}", bufs=1)
                    for h in range(HPC)
                ]
                expts = {}

                def emit_v_group(tg):
                    # V: [t, hk] natural layout, both heads fused per matmul.
                    # One [128,4,512] load = four d-strips of this t-chunk in
                    # a single descriptor, issued from the idle GpSimd queue
                    # (Sync's serial descriptor issue was pacing this phase).
                    psv = [
                        pps.tile(
                            [128, HPC * 128], F32, name=f"psv{tt}", tag=f"ph{tt}", bufs=1
                        )
                        for tt in range(4)
                    ]
                    for qq in range(4):
                        xc4 = xpool.tile(
                            [128, 4, 512], BF16, name="xc4", tag="xc4", bufs=3
                        )
                        nc.gpsimd.dma_start(
                            xc4[:],
                            vT3[:, 4 * qq : 4 * qq + 4, tg * 512 : (tg + 1) * 512],
                        )
                        for dtl in range(4):
                            dt = 4 * qq + dtl
                            for tt in range(4):
                                nc.tensor.matmul(
                                    psv[tt][:],
                                    xc4[:, dtl, tt * 128 : (tt + 1) * 128],
                                    wv2_sb[:, dt * HPC * 128 : (dt + 1) * HPC * 128],
                                    start=(dt == 0),
                                    stop=(dt == NT - 1),
                                )
                    for tt in range(4):
                        t_tile = tg * 4 + tt
                        for h in range(HPC):
                            nc.vector.tensor_tensor(
                                V[h][:, t_tile * 128 : (t_tile + 1) * 128],
                                psv[tt][:, h * 128 : (h + 1) * 128],
                                bvb_sb[:, h * 128 : (h + 1) * 128],
                                op=mybir.AluOpType.add,
                            )

                rects = {}

                def emit_pass1_partA(h, i):
                    # scores -> exp (+row-sum) -> 1/rowsum
                    expt = exppool.tile(
                        [128, S], BF16, name=f"expt{i}", tag=f"exp{i}", bufs=1
                    )
                    expts[(h, i)] = expt
                    sumt = small.tile([128, 2], F32, name="sumt", tag="sum", bufs=4)
                    for half in range(2):
                        psc = pps.tile(
                            [128, 1024], F32, name=f"psc{half}", tag=f"psc{half}", bufs=1
                        )
                        for cc in range(2):
                            c = half * 2 + cc
                            nc.tensor.matmul(
                                psc[:, cc * 512 : (cc + 1) * 512],
                                KT[h][:, i * 128 : (i + 1) * 128],
                                QT[h][:, c * 512 : (c + 1) * 512],
                                start=True,
                                stop=True,
                            )
                        nc.scalar.activation(
                            expt[:, half * 1024 : (half + 1) * 1024],
                            psc[:],
                            AF.Exp,
                            scale=SCALE,
                            accum_out=sumt[:, half : half + 1],
                        )
                    rect = small.tile(
                        [128, 1], F32, name="rect", tag=f"rec{i % 4}", bufs=2
                    )
                    rects[(h, i)] = rect
                    nc.vector.reduce_sum(rect[:], sumt[:], axis=mybir.AxisListType.X)
                    nc.vector.reciprocal(rect[:], rect[:])

                def emit_pass1_partB(h, i):
                    # fold 1/rowsum into this strip's V rows
                    nc.vector.tensor_scalar_mul(
                        vsca[h][:, i * 128 : (i + 1) * 128],
                        V[h][:, i * 128 : (i + 1) * 128],
                        rects[(h, i)][:],
                    )

                def emit_pass1_strip(h, i):
                    emit_pass1_partA(h, i)
                    emit_pass1_partB(h, i)

                def emit_pass2_strip(h, ph, i):
                    for c in range(NSC):
                        nc.tensor.matmul(
                            ph[c][:],
                            vsca[h][:, i * 128 : (i + 1) * 128],
                            expts[(h, i)][:, c * 512 : (c + 1) * 512],
                            start=(i == 0),
                            stop=(i == NT - 1),
                        )

                def emit_pass2(h):
                    ph = [
                        pps.tile([128, 512], F32, name=f"ph{c}", tag=f"ph{c}", bufs=1)
                        for c in range(NSC)
                    ]
                    for i in range(NT):
                        emit_pass2_strip(h, ph, i)
                    for c in range(NSC):
                        nc.vector.tensor_copy(HT[h][:, c * 512 : (c + 1) * 512], ph[c][:])

                _mark(nc, "PV+S0")
                # Interleave: strips' scores+exp first (feed ACT), V matmul
                # group fills PE during the exps, then the vsc muls that need
                # this group's V tiles.
                rects = {}
                for g in range(4):
                    for i in range(4 * g, 4 * g + 4):
                        emit_pass1_partA(0, i)
                    emit_v_group(g)
                    for i in range(4 * g, 4 * g + 4):
                        emit_pass1_partB(0, i)
                _mark(nc, "S2h0+S1h1")
                # pass2 of head 0 (PE-dense) strip-interleaved with pass1 of
                # head 1 (ACT-bound)
                ph0 = [
                    pps.tile([128, 512], F32, name=f"ph{c}", tag=f"ph{c}", bufs=1)
                    for c in range(NSC)
                ]
                for i in range(NT):
                    emit_pass2_strip(0, ph0, i)
                    emit_pass1_partA(1, i)
                    emit_pass1_partB(1, i)
                for c in range(NSC):
                    nc.vector.tensor_copy(HT[0][:, c * 512 : (c + 1) * 512], ph0[c][:])
                _mark(nc, "S2-h1")
                emit_pass2(1)

            # ---------------- phase O: partial output projection --------------
            with tc.tile_pool(name="ppo", bufs=3, space="PSUM") as ppo:
                _mark(nc, "O")
                for st in range(NT):
                    for cp in range(2):
                        po = ppo.tile([128, 1024], F32, name="po", tag="po", bufs=3)
                        for cc in range(2):
                            c = cp * 2 + cc
                            for h in range(HPC):
                                nc.tensor.matmul(
                                    po[:, cc * 512 : (cc + 1) * 512],
                                    HT[h][:, st * 128 : (st + 1) * 128],
                                    wo_sb[:, h * D + c * 512 : h * D + (c + 1) * 512],
                                    start=(h == 0),
                                    stop=(h == HPC - 1),
                                )
                        ot = opool.tile([128, 1024], BF16, name="ot", tag="ot", bufs=4)
                        if cp % 2 == 0:
                            nc.vector.tensor_copy(ot[:], po[:])
                        else:
                            nc.scalar.copy(ot[:], po[:])
                        nc.sync.dma_start(
                            out[st * 128 : (st + 1) * 128, cp * 1024 : (cp + 1) * 1024],
                            ot[:],
                        )

            if loop_ctx is not None:
                loop_ctx.__exit__(None, None, None)

    return nc


_NC = None


def _get_nc():
    global _NC
    if _NC is None:
        _NC = _build_bass()
    return _NC


def _prep_inputs(query, key, value, Wq, bq, Wk, bk, Wv, bv, Wo, bo):
    """Host-side shard + pack. Returns per-core input maps."""
    bf = ml_dtypes.bfloat16
    f32 = np.float32

    query = np.asarray(query, f32)
    key = np.asarray(key, f32)
    value = np.asarray(value, f32)
    Wq = np.asarray(Wq, f32)
    Wk = np.asarray(Wk, f32)
    Wv = np.asarray(Wv, f32)
    Wo = np.asarray(Wo, f32)
    bq = np.asarray(bq, f32)
    bk = np.asarray(bk, f32)
    bv = np.asarray(bv, f32)

    qT = np.ascontiguousarray(query.T).astype(bf)
    kT = np.ascontiguousarray(key.T).astype(bf)
    # vT3[p, dt, s] = value[s, dt*128+p]
    vT3 = np.ascontiguousarray(
        value.T.reshape(NT, 128, S).transpose(1, 0, 2)
    ).astype(bf)

    in_maps = []
    for c in range(N_CORES):
        heads = [c * HPC + j for j in range(HPC)]

        # all stacks dt-major: col = (dt*HPC + h)*128 + k, row = d within tile
        def pack_w(W):
            return np.concatenate(
                [
                    np.concatenate(
                        [W[hh].reshape(NT, 128, DK)[dt] for hh in heads], axis=1
                    )
                    for dt in range(NT)
                ],
                axis=1,
            ).astype(bf)

        wv2 = pack_w(Wv)

        wo_p = np.concatenate(
            [Wo[hh * DK : (hh + 1) * DK, :] for hh in heads], axis=1
        ).astype(bf)

        bqk = np.stack(
            [bq[hh] for hh in heads] + [bk[hh] for hh in heads], axis=1
        ).astype(f32)
        bvb = np.concatenate(
            [np.broadcast_to(bv[hh][None, :], (128, DK)) for hh in heads], axis=1
        ).astype(f32)

        in_maps.append(
            {
                "qT": qT,
                "kT": kT,
                "vT3": vT3,
                "wq": np.ascontiguousarray(pack_w(Wq)),
                "wk": np.ascontiguousarray(pack_w(Wk)),
                "wv2": np.ascontiguousarray(wv2),
                "wo": np.ascontiguousarray(wo_p),
                "bqk": np.ascontiguousarray(bqk),
                "bvb": np.ascontiguousarray(bvb),
            }
        )
    return in_maps


def kernel(query, key, value, Wq, bq, Wk, bk, Wv, bv, Wo, bo):
    global LAST_RESULTS
    in_maps = _prep_inputs(query, key, value, Wq, bq, Wk, bk, Wv, bv, Wo, bo)
    nc = _get_nc()
    res = bass_utils.run_bass_kernel_spmd(
        nc, in_maps, core_ids=list(range(N_CORES)), trace=TRACE
    )
    LAST_RESULTS = res
    acc = res.results[0]["out_p"].astype(np.float32)
    for c in range(1, N_CORES):
        acc += res.results[c]["out_p"].astype(np.float32)
    acc += np.asarray(bo, np.float32)[None, :]
    return acc



# revision 32
# speedup vs baseline: 1.1672x; 1.0033x over previous
"""Multi-head attention (softmax over the query axis) on 8 trn2 cores.

Sharding: tensor-parallel over heads — 2 heads per core. Each core computes
its heads' projections + attention + a partial output projection (row-parallel
Wo); the host sums the 8 partial outputs and adds bo.

Device-side layout choices (host pre-packs everything):
  - activations are shipped TRANSPOSED (d on partitions) as bf16, so every
    matmul contracts over the partition dim with natural-layout DMAs.
  - scores are computed transposed ([t, s]) so the softmax axis (query s) is
    the free axis: one exp-activation per strip with fused row-sum.
  - 1/rowsum is folded into V's rows (16K elems) instead of the attention
    matrix (4.2M elems).
"""

import json

import numpy as np
import ml_dtypes

import concourse.bass as bass
import concourse.mybir as mybir
import concourse.tile as tile
from concourse import bass_utils

BF16 = mybir.dt.bfloat16
F32 = mybir.dt.float32
AF = mybir.ActivationFunctionType

N_CORES = 8
H = 16
D = 2048
DK = 128
S = 2048
HPC = H // N_CORES          # heads per core = 2
NT = D // 128               # 16 tiles along d / t
NSC = S // 512              # 4 chunks of 512 along s / m
SCALE = 1.0 / float(np.sqrt(DK))

TRACE = False
LAST_RESULTS = None
PHASE_MARKS = []


def _mark(nc, label):
    PHASE_MARKS.append((label, nc.next_id()))


# The walrus in this container accepts only ONE sem-wait per instruction
# (setupSyncWait: "Too many sync wait commands"), but Tile attaches one wait
# per depended-on semaphore. Split extra waits onto single-wait NoOps inserted
# just before the instruction on the same engine, at BIR-JSON level so every
# compile path (native + bass2jax/axon) is covered.
def _split_multi_waits(raw: bytes) -> bytes:
    m = json.loads(raw)
    ctr = 0
    changed = False
    for fn in m.get("functions", []):
        for blk in fn.get("blocks", []):
            insts = blk.get("instructions", [])
            out = []
            for inst in insts:
                si = inst.get("sync_info")
                waits = (si.get("on_wait") or []) if si else []
                if len(waits) > 1:
                    changed = True
                    for w in waits[:-1]:
                        ctr += 1
                        out.append(
                            {
                                "debug": inst.get("debug"),
                                "engine": inst["engine"],
                                "ins": [],
                                "name": f"I-wsplit-{ctr}",
                                "opcode": "NoOp",
                                "outs": [],
                                "sync_info": {"on_update": [], "on_wait": [w]},
                            }
                        )
                    si["on_wait"] = [waits[-1]]
                out.append(inst)
            if changed:
                blk["instructions"] = out
    if not changed:
        return raw
    return json.dumps(m).encode()


_orig_to_json_bytes = bass.Bass.to_json_bytes


def _to_json_bytes_split(self):
    return _split_multi_waits(_orig_to_json_bytes(self))


bass.Bass.to_json_bytes = _to_json_bytes_split


def _build_bass(loop_n=None):
    nc = bass.Bass(trn_type="TRN2")

    qT = nc.dram_tensor("qT", [D, S], BF16, kind="ExternalInput")
    kT = nc.dram_tensor("kT", [D, S], BF16, kind="ExternalInput")
    # vT blocked [128, NT, S]: vT3[p, dt, s] = value[s, dt*128+p], so one
    # descriptor fetches four d-strips of a t-chunk
    vT3 = nc.dram_tensor("vT3", [128, NT, S], BF16, kind="ExternalInput")
    wq = nc.dram_tensor("wq", [128, HPC * NT * 128], BF16, kind="ExternalInput")
    wk = nc.dram_tensor("wk", [128, HPC * NT * 128], BF16, kind="ExternalInput")
    wv2 = nc.dram_tensor("wv2", [128, NT * HPC * 128], BF16, kind="ExternalInput")
    wo = nc.dram_tensor("wo", [128, HPC * D], BF16, kind="ExternalInput")
    bqk = nc.dram_tensor("bqk", [128, 2 * HPC], F32, kind="ExternalInput")
    bvb = nc.dram_tensor("bvb", [128, HPC * 128], F32, kind="ExternalInput")
    out = nc.dram_tensor("out_p", [S, D], BF16, kind="ExternalOutput")

    with tile.TileContext(nc) as tc:
        with (
            tc.tile_pool(name="wpool", bufs=1) as wpool,
            tc.tile_pool(name="acts", bufs=1) as acts,
            tc.tile_pool(name="xpool", bufs=4) as xpool,
            tc.tile_pool(name="small", bufs=2) as small,
            tc.tile_pool(name="opool", bufs=2) as opool,
            tc.tile_pool(name="exppool", bufs=1) as exppool,
        ):
            # --- resident weights ---
            wq_sb = wpool.tile([128, HPC * NT * 128], BF16)
            wk_sb = wpool.tile([128, HPC * NT * 128], BF16)
            wv2_sb = wpool.tile([128, NT * HPC * 128], BF16)
            wo_sb = wpool.tile([128, HPC * D], BF16)
            bqk_sb = wpool.tile([128, 2 * HPC], F32)
            bvb_sb = wpool.tile([128, HPC * 128], F32)
            nc.sync.dma_start(wq_sb[:], wq[:])
            nc.sync.dma_start(bqk_sb[:], bqk[:])

            # --- resident per-head activations ---
            QT = [acts.tile([128, S], BF16, name=f"QT{h}") for h in range(HPC)]
            KT = [acts.tile([128, S], BF16, name=f"KT{h}") for h in range(HPC)]
            V = [acts.tile([128, NT * 128], BF16, name=f"V{h}") for h in range(HPC)]
            HT = [acts.tile([128, S], BF16, name=f"HT{h}") for h in range(HPC)]

            # benchmark mode: run the whole body loop_n times in one NEFF to
            # amortize dispatch overhead; weights loaded once up front.
            loop_ctx = None
            if loop_n:
                nc.sync.dma_start(wk_sb[:], wk[:])
                nc.sync.dma_start(wv2_sb[:], wv2[:])
                nc.sync.dma_start(bvb_sb[:], bvb[:])
                nc.sync.dma_start(wo_sb[:], wo[:])
                loop_ctx = tc.For_i(0, loop_n, 1)
                loop_ctx.__enter__()

            # ---------------- phase P-QK: Q^T / K^T projections ---------------
            # Full-strip loads ([128, S] = 4KB lines); one psum bank per
            # (head, s-chunk), accumulated across all 16 d-strips.
            with tc.tile_pool(name="ppqk", bufs=1, space="PSUM") as ppqk:
                for xdram, w_sb, dst, bcol in ((qT, wq_sb, QT, 0), (kT, wk_sb, KT, HPC)):
                    _mark(nc, "P-Q" if xdram is qT else "P-K")
                    if xdram is kT and not loop_n:
                        nc.sync.dma_start(wk_sb[:], wk[:])
                    ps = [
                        [
                            ppqk.tile(
                                [128, 512], F32, name=f"pp{h}{c}", tag=f"pp{h}{c}", bufs=1
                            )
                            for c in range(NSC)
                        ]
                        for h in range(HPC)
                    ]
                    for dt in range(NT):
                        xs = xpool.tile([128, S], BF16, name="xs", tag="xs", bufs=6)
                        nc.sync.dma_start(xs[:], xdram[dt * 128 : (dt + 1) * 128, :])
                        if xdram is qT and dt == 1 and not loop_n:
                            nc.sync.dma_start(wv2_sb[:], wv2[:])
                            nc.sync.dma_start(bvb_sb[:], bvb[:])
                        for h in range(HPC):
                            for c in range(NSC):
                                nc.tensor.matmul(
                                    ps[h][c][:],
                                    w_sb[:, (h * NT + dt) * 128 : (h * NT + dt + 1) * 128],
                                    xs[:, c * 512 : (c + 1) * 512],
                                    start=(dt == 0),
                                    stop=(dt == NT - 1),
                                )
                    for h in range(HPC):
                        for c in range(NSC):
                            nc.scalar.activation(
                                dst[h][:, c * 512 : (c + 1) * 512],
                                ps[h][c][:],
                                AF.Identity,
                                bias=bqk_sb[:, bcol + h : bcol + h + 1],
                                scale=1.0,
                            )

            # ------- phases P-V and S interleaved on one psum pool -----------
            # P-V shares the ph* psum tags (1 bank each) with S pass 2; the V
            # matmul groups are interleaved with S-h0 pass 1 so the PE stays
            # busy under pass 1's ACT-bound exp stream.
            with tc.tile_pool(name="pps", bufs=1, space="PSUM") as pps:
                if not loop_n:
                    nc.sync.dma_start(wo_sb[:], wo[:])
                vsca = [
                    small.tile([128, NT * 128], BF16, name=f"vsca{h}", tag=f"vsca{h}", bufs=1)
                    for h in range(HPC)
                ]
                expts = {}

                def emit_v_pack(tg, qq, psv):
                    # V: [t, hk] natural layout, both heads fused per matmul.
                    # One [128,4,512] load = four d-strips of this t-chunk in
                    # a single descriptor, issued from the idle GpSimd queue.
                    xc4 = xpool.tile(
                        [128, 4, 512], BF16, name="xc4", tag="xc4", bufs=3
                    )
                    nc.gpsimd.dma_start(
                        xc4[:],
                        vT3[:, 4 * qq : 4 * qq + 4, tg * 512 : (tg + 1) * 512],
                    )
                    for dtl in range(4):
                        dt = 4 * qq + dtl
                        for tt in range(4):
                            nc.tensor.matmul(
                                psv[tt][:],
                                xc4[:, dtl, tt * 128 : (tt + 1) * 128],
                                wv2_sb[:, dt * HPC * 128 : (dt + 1) * HPC * 128],
                                start=(dt == 0),
                                stop=(dt == NT - 1),
                            )

                def emit_v_bias(tg, psv):
                    for tt in range(4):
                        t_tile = tg * 4 + tt
                        for h in range(HPC):
                            nc.vector.tensor_tensor(
                                V[h][:, t_tile * 128 : (t_tile + 1) * 128],
                                psv[tt][:, h * 128 : (h + 1) * 128],
                                bvb_sb[:, h * 128 : (h + 1) * 128],
                                op=mybir.AluOpType.add,
                            )

                rects = {}

                def emit_pass1_partA(h, i):
                    # scores -> exp (+row-sum) -> 1/rowsum
                    expt = exppool.tile(
                        [128, S], BF16, name=f"expt{i}", tag=f"exp{i}", bufs=1
                    )
                    expts[(h, i)] = expt
                    sumt = small.tile([128, 2], F32, name="sumt", tag="sum", bufs=4)
                    for half in range(2):
                        psc = pps.tile(
                            [128, 1024], F32, name=f"psc{half}", tag=f"psc{half}", bufs=1
                        )
                        for cc in range(2):
                            c = half * 2 + cc
                            nc.tensor.matmul(
                                psc[:, cc * 512 : (cc + 1) * 512],
                                KT[h][:, i * 128 : (i + 1) * 128],
                                QT[h][:, c * 512 : (c + 1) * 512],
                                start=True,
                                stop=True,
                            )
                        nc.scalar.activation(
                            expt[:, half * 1024 : (half + 1) * 1024],
                            psc[:],
                            AF.Exp,
                            scale=SCALE,
                            accum_out=sumt[:, half : half + 1],
                        )
                    rect = small.tile(
                        [128, 1], F32, name="rect", tag=f"rec{i % 4}", bufs=2
                    )
                    rects[(h, i)] = rect
                    nc.vector.reduce_sum(rect[:], sumt[:], axis=mybir.AxisListType.X)
                    nc.vector.reciprocal(rect[:], rect[:])

                def emit_pass1_partB(h, i):
                    # fold 1/rowsum into this strip's V rows
                    nc.vector.tensor_scalar_mul(
                        vsca[h][:, i * 128 : (i + 1) * 128],
                        V[h][:, i * 128 : (i + 1) * 128],
                        rects[(h, i)][:],
                    )

                def emit_pass1_strip(h, i):
                    emit_pass1_partA(h, i)
                    emit_pass1_partB(h, i)

                def emit_pass2_strip(h, ph, i):
                    for c in range(NSC):
                        nc.tensor.matmul(
                            ph[c][:],
                            vsca[h][:, i * 128 : (i + 1) * 128],
                            expts[(h, i)][:, c * 512 : (c + 1) * 512],
                            start=(i == 0),
                            stop=(i == NT - 1),
                        )

                def emit_pass2(h):
                    ph = [
                        pps.tile([128, 512], F32, name=f"ph{c}", tag=f"ph{c}", bufs=1)
                        for c in range(NSC)
                    ]
                    for i in range(NT):
                        emit_pass2_strip(h, ph, i)
                    for c in range(NSC):
                        nc.vector.tensor_copy(HT[h][:, c * 512 : (c + 1) * 512], ph[c][:])

                _mark(nc, "PV+S0")
                # Interleave at pack granularity: each score strip is followed
                # by a 4-dt V pack, so the PE never idles on the psc WAR while
                # the exp stream drains (strip-run serialization was pacing
                # this phase).
                rects = {}
                for g in range(4):
                    psv = [
                        pps.tile(
                            [128, HPC * 128], F32, name=f"psv{tt}", tag=f"ph{tt}",
                            bufs=1,
                        )
                        for tt in range(4)
                    ]
                    for q in range(4):
                        emit_pass1_partA(0, 4 * g + q)
                        emit_v_pack(g, q, psv)
                    emit_v_bias(g, psv)
                    for i in range(4 * g, 4 * g + 4):
                        emit_pass1_partB(0, i)
                _mark(nc, "S2h0+S1h1")
                # pass2 of head 0 (PE-dense) strip-interleaved with pass1 of
                # head 1 (ACT-bound)
                ph0 = [
                    pps.tile([128, 512], F32, name=f"ph{c}", tag=f"ph{c}", bufs=1)
                    for c in range(NSC)
                ]
                for i in range(NT):
                    emit_pass2_strip(0, ph0, i)
                    emit_pass1_partA(1, i)
                    emit_pass1_partB(1, i)
                for c in range(NSC):
                    nc.vector.tensor_copy(HT[0][:, c * 512 : (c + 1) * 512], ph0[c][:])
                _mark(nc, "S2-h1")
                emit_pass2(1)

            # ---------------- phase O: partial output projection --------------
            with tc.tile_pool(name="ppo", bufs=3, space="PSUM") as ppo:
                _mark(nc, "O")
                for st in range(NT):
                    for cp in range(2):
                        po = ppo.tile([128, 1024], F32, name="po", tag="po", bufs=3)
                        for cc in range(2):
                            c = cp * 2 + cc
                            for h in range(HPC):
                                nc.tensor.matmul(
                                    po[:, cc * 512 : (cc + 1) * 512],
                                    HT[h][:, st * 128 : (st + 1) * 128],
                                    wo_sb[:, h * D + c * 512 : h * D + (c + 1) * 512],
                                    start=(h == 0),
                                    stop=(h == HPC - 1),
                                )
                        ot = opool.tile([128, 1024], BF16, name="ot", tag="ot", bufs=4)
                        if cp % 2 == 0:
                            nc.vector.tensor_copy(ot[:], po[:])
                        else:
                            nc.scalar.copy(ot[:], po[:])
                        nc.sync.dma_start(
                            out[st * 128 : (st + 1) * 128, cp * 1024 : (cp + 1) * 1024],
                            ot[:],
                        )

            if loop_ctx is not None:
                loop_ctx.__exit__(None, None, None)

    return nc


_NC = None


def _get_nc():
    global _NC
    if _NC is None:
        _NC = _build_bass()
    return _NC


def _prep_inputs(query, key, value, Wq, bq, Wk, bk, Wv, bv, Wo, bo):
    """Host-side shard + pack. Returns per-core input maps."""
    bf = ml_dtypes.bfloat16
    f32 = np.float32

    query = np.asarray(query, f32)
    key = np.asarray(key, f32)
    value = np.asarray(value, f32)
    Wq = np.asarray(Wq, f32)
    Wk = np.asarray(Wk, f32)
    Wv = np.asarray(Wv, f32)
    Wo = np.asarray(Wo, f32)
    bq = np.asarray(bq, f32)
    bk = np.asarray(bk, f32)
    bv = np.asarray(bv, f32)

    qT = np.ascontiguousarray(query.T).astype(bf)
    kT = np.ascontiguousarray(key.T).astype(bf)
    # vT3[p, dt, s] = value[s, dt*128+p]
    vT3 = np.ascontiguousarray(
        value.T.reshape(NT, 128, S).transpose(1, 0, 2)
    ).astype(bf)

    in_maps = []
    for c in range(N_CORES):
        heads = [c * HPC + j for j in range(HPC)]
        # wq/wk: [128, h*NT*128], col = (h*NT + dt)*128 + k, row = d within tile
        def pack_w(W):
            blocks = [
                W[hh].reshape(NT, 128, DK).transpose(1, 0, 2).reshape(128, NT * DK)
                for hh in heads
            ]
            return np.concatenate(blocks, axis=1).astype(bf)

        # wv2: [128, NT*HPC*128], col = dt*(HPC*128) + h*128 + k
        wv2 = np.concatenate(
            [
                np.concatenate([Wv[hh].reshape(NT, 128, DK)[dt] for hh in heads], axis=1)
                for dt in range(NT)
            ],
            axis=1,
        ).astype(bf)

        wo_p = np.concatenate(
            [Wo[hh * DK : (hh + 1) * DK, :] for hh in heads], axis=1
        ).astype(bf)

        bqk = np.stack(
            [bq[hh] for hh in heads] + [bk[hh] for hh in heads], axis=1
        ).astype(f32)
        bvb = np.concatenate(
            [np.broadcast_to(bv[hh][None, :], (128, DK)) for hh in heads], axis=1
        ).astype(f32)

        in_maps.append(
            {
                "qT": qT,
                "kT": kT,
                "vT3": vT3,
                "wq": pack_w(Wq),
                "wk": pack_w(Wk),
                "wv2": np.ascontiguousarray(wv2),
                "wo": np.ascontiguousarray(wo_p),
                "bqk": np.ascontiguousarray(bqk),
                "bvb": np.ascontiguousarray(bvb),
            }
        )
    return in_maps


def kernel(query, key, value, Wq, bq, Wk, bk, Wv, bv, Wo, bo):
    global LAST_RESULTS
    in_maps = _prep_inputs(query, key, value, Wq, bq, Wk, bk, Wv, bv, Wo, bo)
    nc = _get_nc()
    res = bass_utils.run_bass_kernel_spmd(
        nc, in_maps, core_ids=list(range(N_CORES)), trace=TRACE
    )
    LAST_RESULTS = res
    acc = res.results[0]["out_p"].astype(np.float32)
    for c in range(1, N_CORES):
        acc += res.results[c]["out_p"].astype(np.float32)
    acc += np.asarray(bo, np.float32)[None, :]
    return acc



# revision 33
# speedup vs baseline: 1.1992x; 1.0274x over previous
"""Multi-head attention (softmax over the query axis) on 8 trn2 cores.

Sharding: tensor-parallel over heads — 2 heads per core. Each core computes
its heads' projections + attention + a partial output projection (row-parallel
Wo); the host sums the 8 partial outputs and adds bo.

Device-side layout choices (host pre-packs everything):
  - activations are shipped TRANSPOSED (d on partitions) as bf16, so every
    matmul contracts over the partition dim with natural-layout DMAs.
  - scores are computed transposed ([t, s]) so the softmax axis (query s) is
    the free axis: one exp-activation per strip with fused row-sum.
  - 1/rowsum is folded into V's rows (16K elems) instead of the attention
    matrix (4.2M elems).
"""

import json

import numpy as np
import ml_dtypes

import concourse.bass as bass
import concourse.mybir as mybir
import concourse.tile as tile
from concourse import bass_utils

BF16 = mybir.dt.bfloat16
F32 = mybir.dt.float32
AF = mybir.ActivationFunctionType

N_CORES = 8
H = 16
D = 2048
DK = 128
S = 2048
HPC = H // N_CORES          # heads per core = 2
NT = D // 128               # 16 tiles along d / t
NSC = S // 512              # 4 chunks of 512 along s / m
SCALE = 1.0 / float(np.sqrt(DK))

TRACE = False
LAST_RESULTS = None
PHASE_MARKS = []


def _mark(nc, label):
    PHASE_MARKS.append((label, nc.next_id()))


# The walrus in this container accepts only ONE sem-wait per instruction
# (setupSyncWait: "Too many sync wait commands"), but Tile attaches one wait
# per depended-on semaphore. Split extra waits onto single-wait NoOps inserted
# just before the instruction on the same engine, at BIR-JSON level so every
# compile path (native + bass2jax/axon) is covered.
def _split_multi_waits(raw: bytes) -> bytes:
    m = json.loads(raw)
    ctr = 0
    changed = False
    for fn in m.get("functions", []):
        for blk in fn.get("blocks", []):
            insts = blk.get("instructions", [])
            out = []
            for inst in insts:
                si = inst.get("sync_info")
                waits = (si.get("on_wait") or []) if si else []
                if len(waits) > 1:
                    changed = True
                    for w in waits[:-1]:
                        ctr += 1
                        out.append(
                            {
                                "debug": inst.get("debug"),
                                "engine": inst["engine"],
                                "ins": [],
                                "name": f"I-wsplit-{ctr}",
                                "opcode": "NoOp",
                                "outs": [],
                                "sync_info": {"on_update": [], "on_wait": [w]},
                            }
                        )
                    si["on_wait"] = [waits[-1]]
                out.append(inst)
            if changed:
                blk["instructions"] = out
    if not changed:
        return raw
    return json.dumps(m).encode()


_orig_to_json_bytes = bass.Bass.to_json_bytes


def _to_json_bytes_split(self):
    return _split_multi_waits(_orig_to_json_bytes(self))


bass.Bass.to_json_bytes = _to_json_bytes_split


def _build_bass(loop_n=None):
    nc = bass.Bass(trn_type="TRN2")

    qT = nc.dram_tensor("qT", [D, S], BF16, kind="ExternalInput")
    kT = nc.dram_tensor("kT", [D, S], BF16, kind="ExternalInput")
    vT = nc.dram_tensor("vT", [D, S], BF16, kind="ExternalInput")
    wq = nc.dram_tensor("wq", [128, HPC * NT * 128], BF16, kind="ExternalInput")
    wk = nc.dram_tensor("wk", [128, HPC * NT * 128], BF16, kind="ExternalInput")
    wv2 = nc.dram_tensor("wv2", [128, NT * HPC * 128], BF16, kind="ExternalInput")
    wo = nc.dram_tensor("wo", [128, HPC * D], BF16, kind="ExternalInput")
    bqk = nc.dram_tensor("bqk", [128, 2 * HPC], F32, kind="ExternalInput")
    bvb = nc.dram_tensor("bvb", [128, HPC * 128], F32, kind="ExternalInput")
    out = nc.dram_tensor("out_p", [S, D], BF16, kind="ExternalOutput")

    with tile.TileContext(nc) as tc:
        with (
            tc.tile_pool(name="wpool", bufs=1) as wpool,
            tc.tile_pool(name="acts", bufs=1) as acts,
            tc.tile_pool(name="xpool", bufs=4) as xpool,
            tc.tile_pool(name="small", bufs=2) as small,
            tc.tile_pool(name="opool", bufs=2) as opool,
            tc.tile_pool(name="exppool", bufs=1) as exppool,
        ):
            # --- resident weights ---
            wq_sb = wpool.tile([128, HPC * NT * 128], BF16)
            wk_sb = wpool.tile([128, HPC * NT * 128], BF16)
            wv2_sb = wpool.tile([128, NT * HPC * 128], BF16)
            wo_sb = wpool.tile([128, HPC * D], BF16)
            bqk_sb = wpool.tile([128, 2 * HPC], F32)
            bvb_sb = wpool.tile([128, HPC * 128], F32)
            nc.sync.dma_start(wq_sb[:], wq[:])
            nc.sync.dma_start(bqk_sb[:], bqk[:])

            # --- resident per-head activations ---
            QT = [acts.tile([128, S], BF16, name=f"QT{h}") for h in range(HPC)]
            KT = [acts.tile([128, S], BF16, name=f"KT{h}") for h in range(HPC)]
            V = [acts.tile([128, NT * 128], BF16, name=f"V{h}") for h in range(HPC)]
            HT = [acts.tile([128, S], BF16, name=f"HT{h}") for h in range(HPC)]

            # benchmark mode: run the whole body loop_n times in one NEFF to
            # amortize dispatch overhead; weights loaded once up front.
            loop_ctx = None
            if loop_n:
                nc.sync.dma_start(wk_sb[:], wk[:])
                nc.sync.dma_start(wv2_sb[:], wv2[:])
                nc.sync.dma_start(bvb_sb[:], bvb[:])
                nc.sync.dma_start(wo_sb[:], wo[:])
                loop_ctx = tc.For_i(0, loop_n, 1)
                loop_ctx.__enter__()

            # ---------------- phase P-QK: Q^T / K^T projections ---------------
            # Full-strip loads ([128, S] = 4KB lines); one psum bank per
            # (head, s-chunk), accumulated across all 16 d-strips.
            with tc.tile_pool(name="ppqk", bufs=1, space="PSUM") as ppqk:
                for xdram, w_sb, dst, bcol in ((qT, wq_sb, QT, 0), (kT, wk_sb, KT, HPC)):
                    _mark(nc, "P-Q" if xdram is qT else "P-K")
                    if xdram is kT and not loop_n:
                        nc.sync.dma_start(wk_sb[:], wk[:])
                    ps = [
                        [
                            ppqk.tile(
                                [128, 512], F32, name=f"pp{h}{c}", tag=f"pp{h}{c}", bufs=1
                            )
                            for c in range(NSC)
                        ]
                        for h in range(HPC)
                    ]
                    for dt in range(NT):
                        xs = xpool.tile([128, S], BF16, name="xs", tag="xs", bufs=6)
                        nc.sync.dma_start(xs[:], xdram[dt * 128 : (dt + 1) * 128, :])
                        if xdram is qT and dt == 1 and not loop_n:
                            nc.sync.dma_start(wv2_sb[:], wv2[:])
                            nc.sync.dma_start(bvb_sb[:], bvb[:])
                        for h in range(HPC):
                            for c in range(NSC):
                                nc.tensor.matmul(
                                    ps[h][c][:],
                                    w_sb[:, (h * NT + dt) * 128 : (h * NT + dt + 1) * 128],
                                    xs[:, c * 512 : (c + 1) * 512],
                                    start=(dt == 0),
                                    stop=(dt == NT - 1),
                                )
                    for h in range(HPC):
                        for c in range(NSC):
                            nc.scalar.activation(
                                dst[h][:, c * 512 : (c + 1) * 512],
                                ps[h][c][:],
                                AF.Identity,
                                bias=bqk_sb[:, bcol + h : bcol + h + 1],
                                scale=1.0,
                            )

            # ------- phases P-V and S interleaved on one psum pool -----------
            # P-V shares the ph* psum tags (1 bank each) with S pass 2; the V
            # matmul groups are interleaved with S-h0 pass 1 so the PE stays
            # busy under pass 1's ACT-bound exp stream.
            with tc.tile_pool(name="pps", bufs=1, space="PSUM") as pps:
                if not loop_n:
                    nc.sync.dma_start(wo_sb[:], wo[:])
                vsca = [
                    small.tile([128, NT * 128], BF16, name=f"vsca{h}", tag=f"vsca{h}", bufs=1)
                    for h in range(HPC)
                ]
                expts = {}

                def emit_v_group(tg):
                    # V: [t, hk] natural layout, both heads fused per matmul
                    psv = [
                        pps.tile(
                            [128, HPC * 128], F32, name=f"psv{tt}", tag=f"ph{tt}", bufs=1
                        )
                        for tt in range(4)
                    ]
                    for dt in range(NT):
                        xc = xpool.tile([128, 512], BF16, name="xc", tag="xc", bufs=12)
                        nc.sync.dma_start(
                            xc[:], vT[dt * 128 : (dt + 1) * 128, tg * 512 : (tg + 1) * 512]
                        )
                        for tt in range(4):
                            nc.tensor.matmul(
                                psv[tt][:],
                                xc[:, tt * 128 : (tt + 1) * 128],
                                wv2_sb[:, dt * HPC * 128 : (dt + 1) * HPC * 128],
                                start=(dt == 0),
                                stop=(dt == NT - 1),
                            )
                    for tt in range(4):
                        t_tile = tg * 4 + tt
                        for h in range(HPC):
                            nc.vector.tensor_tensor(
                                V[h][:, t_tile * 128 : (t_tile + 1) * 128],
                                psv[tt][:, h * 128 : (h + 1) * 128],
                                bvb_sb[:, h * 128 : (h + 1) * 128],
                                op=mybir.AluOpType.add,
                            )

                rects = {}

                def emit_pass1_partA(h, i):
                    # scores -> exp (+row-sum) -> 1/rowsum
                    expt = exppool.tile(
                        [128, S], BF16, name=f"expt{i}", tag=f"exp{i}", bufs=1
                    )
                    expts[(h, i)] = expt
                    sumt = small.tile([128, 2], F32, name="sumt", tag="sum", bufs=4)
                    for half in range(2):
                        psc = pps.tile(
                            [128, 1024], F32, name=f"psc{half}", tag=f"psc{half}", bufs=1
                        )
                        for cc in range(2):
                            c = half * 2 + cc
                            nc.tensor.matmul(
                                psc[:, cc * 512 : (cc + 1) * 512],
                                KT[h][:, i * 128 : (i + 1) * 128],
                                QT[h][:, c * 512 : (c + 1) * 512],
                                start=True,
                                stop=True,
                            )
                        nc.scalar.activation(
                            expt[:, half * 1024 : (half + 1) * 1024],
                            psc[:],
                            AF.Exp,
                            scale=SCALE,
                            accum_out=sumt[:, half : half + 1],
                        )
                    rect = small.tile(
                        [128, 1], F32, name="rect", tag=f"rec{i % 4}", bufs=2
                    )
                    rects[(h, i)] = rect
                    nc.vector.reduce_sum(rect[:], sumt[:], axis=mybir.AxisListType.X)
                    nc.vector.reciprocal(rect[:], rect[:])

                def emit_pass1_partB(h, i):
                    # fold 1/rowsum into this strip's V rows
                    nc.vector.tensor_scalar_mul(
                        vsca[h][:, i * 128 : (i + 1) * 128],
                        V[h][:, i * 128 : (i + 1) * 128],
                        rects[(h, i)][:],
                    )

                def emit_pass1_strip(h, i):
                    emit_pass1_partA(h, i)
                    emit_pass1_partB(h, i)

                def emit_pass2_strip(h, ph, i):
                    for c in range(NSC):
                        nc.tensor.matmul(
                            ph[c][:],
                            vsca[h][:, i * 128 : (i + 1) * 128],
                            expts[(h, i)][:, c * 512 : (c + 1) * 512],
                            start=(i == 0),
                            stop=(i == NT - 1),
                        )

                def emit_pass2(h):
                    ph = [
                        pps.tile([128, 512], F32, name=f"ph{c}", tag=f"ph{c}", bufs=1)
                        for c in range(NSC)
                    ]
                    for i in range(NT):
                        emit_pass2_strip(h, ph, i)
                    for c in range(NSC):
                        nc.vector.tensor_copy(HT[h][:, c * 512 : (c + 1) * 512], ph[c][:])

                _mark(nc, "PV+S0")
                # Interleave: strips' scores+exp first (feed ACT), V matmul
                # group fills PE during the exps, then the vsc muls that need
                # this group's V tiles.
                rects = {}
                for g in range(4):
                    for i in range(4 * g, 4 * g + 4):
                        emit_pass1_partA(0, i)
                    emit_v_group(g)
                    for i in range(4 * g, 4 * g + 4):
                        emit_pass1_partB(0, i)
                _mark(nc, "S2h0+S1h1")
                # pass2 of head 0 (PE-dense) strip-interleaved with pass1 of
                # head 1 (ACT-bound)
                ph0 = [
                    pps.tile([128, 512], F32, name=f"ph{c}", tag=f"ph{c}", bufs=1)
                    for c in range(NSC)
                ]
                for i in range(NT):
                    emit_pass2_strip(0, ph0, i)
                    emit_pass1_partA(1, i)
                    emit_pass1_partB(1, i)
                for c in range(NSC):
                    nc.vector.tensor_copy(HT[0][:, c * 512 : (c + 1) * 512], ph0[c][:])
                _mark(nc, "S2-h1")
                emit_pass2(1)

            # ---------------- phase O: partial output projection --------------
            with tc.tile_pool(name="ppo", bufs=3, space="PSUM") as ppo:
                _mark(nc, "O")
                for st in range(NT):
                    for cp in range(2):
                        po = ppo.tile([128, 1024], F32, name="po", tag="po", bufs=3)
                        for cc in range(2):
                            c = cp * 2 + cc
                            for h in range(HPC):
                                nc.tensor.matmul(
                                    po[:, cc * 512 : (cc + 1) * 512],
                                    HT[h][:, st * 128 : (st + 1) * 128],
                                    wo_sb[:, h * D + c * 512 : h * D + (c + 1) * 512],
                                    start=(h == 0),
                                    stop=(h == HPC - 1),
                                )
                        ot = opool.tile([128, 1024], BF16, name="ot", tag="ot", bufs=4)
                        if cp % 2 == 0:
                            nc.vector.tensor_copy(ot[:], po[:])
                        else:
                            nc.scalar.copy(ot[:], po[:])
                        nc.sync.dma_start(
                            out[st * 128 : (st + 1) * 128, cp * 1024 : (cp + 1) * 1024],
                            ot[:],
                        )

            if loop_ctx is not None:
                loop_ctx.__exit__(None, None, None)

    return nc


_NC = None


def _get_nc():
    global _NC
    if _NC is None:
        _NC = _build_bass()
    return _NC


def _prep_inputs(query, key, value, Wq, bq, Wk, bk, Wv, bv, Wo, bo):
    """Host-side shard + pack. Returns per-core input maps."""
    bf = ml_dtypes.bfloat16
    f32 = np.float32

    query = np.asarray(query, f32)
    key = np.asarray(key, f32)
    value = np.asarray(value, f32)
    Wq = np.asarray(Wq, f32)
    Wk = np.asarray(Wk, f32)
    Wv = np.asarray(Wv, f32)
    Wo = np.asarray(Wo, f32)
    bq = np.asarray(bq, f32)
    bk = np.asarray(bk, f32)
    bv = np.asarray(bv, f32)

    qT = np.ascontiguousarray(query.T).astype(bf)
    kT = np.ascontiguousarray(key.T).astype(bf)
    vT = np.ascontiguousarray(value.T).astype(bf)

    in_maps = []
    for c in range(N_CORES):
        heads = [c * HPC + j for j in range(HPC)]
        # wq/wk: [128, h*NT*128], col = (h*NT + dt)*128 + k, row = d within tile
        def pack_w(W):
            blocks = [
                W[hh].reshape(NT, 128, DK).transpose(1, 0, 2).reshape(128, NT * DK)
                for hh in heads
            ]
            return np.concatenate(blocks, axis=1).astype(bf)

        # wv2: [128, NT*HPC*128], col = dt*(HPC*128) + h*128 + k
        wv2 = np.concatenate(
            [
                np.concatenate([Wv[hh].reshape(NT, 128, DK)[dt] for hh in heads], axis=1)
                for dt in range(NT)
            ],
            axis=1,
        ).astype(bf)

        wo_p = np.concatenate(
            [Wo[hh * DK : (hh + 1) * DK, :] for hh in heads], axis=1
        ).astype(bf)

        bqk = np.stack(
            [bq[hh] for hh in heads] + [bk[hh] for hh in heads], axis=1
        ).astype(f32)
        bvb = np.concatenate(
            [np.broadcast_to(bv[hh][None, :], (128, DK)) for hh in heads], axis=1
        ).astype(f32)

        in_maps.append(
            {
                "qT": qT,
                "kT": kT,
                "vT": vT,
                "wq": pack_w(Wq),
                "wk": pack_w(Wk),
                "wv2": np.ascontiguousarray(wv2),
                "wo": np.ascontiguousarray(wo_p),
                "bqk": np.ascontiguousarray(bqk),
                "bvb": np.ascontiguousarray(bvb),
            }
        )
    return in_maps


def kernel(query, key, value, Wq, bq, Wk, bk, Wv, bv, Wo, bo):
    global LAST_RESULTS
    in_maps = _prep_inputs(query, key, value, Wq, bq, Wk, bk, Wv, bv, Wo, bo)
    nc = _get_nc()
    res = bass_utils.run_bass_kernel_spmd(
        nc, in_maps, core_ids=list(range(N_CORES)), trace=TRACE
    )
    LAST_RESULTS = res
    acc = res.results[0]["out_p"].astype(np.float32)
    for c in range(1, N_CORES):
        acc += res.results[c]["out_p"].astype(np.float32)
    acc += np.asarray(bo, np.float32)[None, :]
    return acc

